# revision 1
# baseline (speedup 1.0000x reference)
"""VQ-codebook linear layer on 8 Trainium2 NeuronCores.

Computes  out = x @ W^T + bias  where  W = centroids[labels].reshape(4096, 4096).

Strategy (tensor-parallel over out_features, per the sharding hint:
"shard labels/centroid-gathered weight along out_features, replicate x"):
  - each core owns a 512-row slice of W (out_features / 8)
  - x is replicated to every core (transposed to [in, tok] fp16 on host)
  - the centroid-gathered weight shard is either pre-gathered on host
    (DEVICE_GATHER=False, fast) or dequantized on-device on the GPSIMD
    engine via ap_gather from a per-partition centroid-column table
    (DEVICE_GATHER=True; measured ~1 ms slower — ap_gather runs at
    ~60 cycles/index and cannot hide under the matmul stream)
  - matmul in fp16 (same PE rate as bf16, ~8x less rounding error for
    unit-scale data; fp32 PSUM accumulation): lhsT = W^T tile
    [128 in, 128 out] (stationary), rhs = x^T tile [128 in, 512 tok]
    (moving), PSUM accumulates over the 32 k-tiles; bias added on the
    PSUM->SBUF copy (DVE tensor_scalar_add)
  - per-core output is [512 out, 8192 tok]; host concatenates and
    transposes back to [4, 2048, 4096]
"""

import numpy as np
import ml_dtypes
import concourse.bass as bass
import concourse.tile as tile
from concourse import bacc, mybir, library_config
from concourse import bass_utils

TOK = 8192          # 4 * 2048 tokens
DIN = 4096
DOUT = 4096
BD = 16             # block dim (centroid vector length)
NCLUST = 256        # codebook size
N_CORES = 8
OSH = DOUT // N_CORES        # 512 out features per core
MT = OSH // 128              # 4 m-tiles per core
KT = DIN // 128              # 32 k-tiles
NTOK = 512                   # tokens per matmul (moving free dim)
NT = TOK // NTOK             # 16 n-tiles

DEVICE_GATHER = False  # host-gathered W^T shard: ~243 us/core vs ~938 us
                       # with on-device ap_gather dequant (measured)
PRECISION = "f16"      # "f16" | "bf16" — low-precision matmul dtype


OUT_F16 = True         # write the [512, 8192] output shard as fp16 (host
                       # upconverts): 8 MiB/core less device DMA and ~2.5 s
                       # less D2H per call; costs ~1.3e-3 extra rel err
XSPLIT = 1             # k-splitting the x n-tile DMA measured slower
                       # (xsplit=2 regressed ~0.6 ms/iter in the A/B); keep 1
WREUSE = False         # issue each W^T k-tile's matmul for 2 token-tiles
                       # back-to-back (halves LDWEIGHTS count)


def build_nc(device_gather: bool = DEVICE_GATHER, repeat: int = 1,
             precision: str = PRECISION, out_f16: bool = OUT_F16,
             xsplit: int = XSPLIT, wreuse: bool = WREUSE):
    """Build and bacc-compile the per-core bass program (SPMD: all cores run
    the same program on different DRAM inputs)."""
    import contextlib

    nc = bacc.Bacc("TRN2", target_bir_lowering=False, debug=False,
                   enable_asserts=True, num_devices=N_CORES)
    f32, i16 = mybir.dt.float32, mybir.dt.int16
    bf16 = mybir.dt.float16 if precision == "f16" else mybir.dt.bfloat16

    xT_ap = nc.dram_tensor("xT", [DIN, TOK], bf16, kind="ExternalInput").ap()
    bias_ap = nc.dram_tensor("biasc", [128, MT], f32, kind="ExternalInput").ap()
    if device_gather:
        table_ap = nc.dram_tensor("table", [128, NCLUST], f32,
                                  kind="ExternalInput").ap()
        idx_ap = nc.dram_tensor("idx", [128, MT * KT * 128 // 16], i16,
                                kind="ExternalInput").ap()
    else:
        wt_ap = nc.dram_tensor("wt", [128, MT * KT * 128], bf16,
                               kind="ExternalInput").ap()
    odt = bf16 if out_f16 else f32
    out_ap = nc.dram_tensor("out", [OSH, TOK], odt, kind="ExternalOutput").ap()

    # DRAM views
    # xT [DIN, TOK] -> [128 p, KT, TOK]
    xview = xT_ap.rearrange("(kt p) t -> p kt t", p=128)
    # out [OSH, TOK] -> [MT, 128 p, TOK]
    oview = out_ap.rearrange("(mt p) t -> mt p t", p=128)

    with tile.TileContext(nc) as tc:
        with contextlib.ExitStack() as ctx:
            const_pool = ctx.enter_context(tc.tile_pool(name="const", bufs=1))
            wt_pool = ctx.enter_context(tc.tile_pool(name="wt", bufs=1))
            x_pool = ctx.enter_context(tc.tile_pool(name="x", bufs=2))
            psum_pool = ctx.enter_context(
                tc.tile_pool(name="psum", bufs=4, space="PSUM"))
            out_pool = ctx.enter_context(tc.tile_pool(name="ob", bufs=3))
            if device_gather:
                g_pool = ctx.enter_context(tc.tile_pool(name="g", bufs=2))
                nc.gpsimd.load_library(library_config.ap_gather)

            bias_t = const_pool.tile([128, MT], f32)
            nc.sync.dma_start(bias_t[:], bias_ap[:])

            # W^T per-core shard, bf16, free layout (m, kt, o'): 32 KB/part
            wt_t = [wt_pool.tile([128, KT * 128], bf16, tag=f"wt{m}",
                                 name=f"wt{m}")
                    for m in range(MT)]

            if device_gather:
                table_t = const_pool.tile([128, NCLUST], f32)
                nc.sync.dma_start(table_t[:], table_ap[:])
                idx_t = const_pool.tile([128, MT * KT * 128 // 16], i16)
                nc.sync.dma_start(idx_t[:], idx_ap[:])

            def dequant():
                if device_gather:
                    nidx_chunk = KT * 128  # 4096 per 16-part group per m-chunk
                    for m in range(MT):
                        g = g_pool.tile([128, nidx_chunk], f32, tag="g")
                        nc.gpsimd.ap_gather(
                            g[:], table_t[:],
                            idx_t[:, bass.ts(m, nidx_chunk // 16)],
                            channels=128, num_elems=NCLUST, d=1,
                            num_idxs=nidx_chunk,
                        )
                        nc.vector.tensor_copy(wt_t[m][:], g[:])
                else:
                    for m in range(MT):
                        nc.sync.dma_start(wt_t[m][:],
                                          wt_ap[:, bass.ts(m, KT * 128)])

            ksub = KT // xsplit

            def body_wreuse():
                # pairs of token-tiles share each loaded W^T k-tile: the two
                # matmuls after one implicit LDWEIGHTS reuse the stationary
                # operand, halving weight-load pressure on the PE
                for ng in range(NT // 2):
                    xa = x_pool.tile([128, KT, NTOK], bf16, tag="xn0",
                                     name="xa")
                    nc.sync.dma_start(xa[:], xview[:, :, bass.ts(2 * ng, NTOK)])
                    xb = x_pool.tile([128, KT, NTOK], bf16, tag="xn1",
                                     name="xb")
                    nc.sync.dma_start(xb[:], xview[:, :, bass.ts(2 * ng + 1, NTOK)])
                    for m in range(MT):
                        psa = psum_pool.tile([128, NTOK], f32, tag="ps")
                        psb = psum_pool.tile([128, NTOK], f32, tag="ps")
                        for kt in range(KT):
                            nc.tensor.matmul(
                                psa[:], lhsT=wt_t[m][:, bass.ts(kt, 128)],
                                rhs=xa[:, kt, :],
                                start=(kt == 0), stop=(kt == KT - 1))
                            nc.tensor.matmul(
                                psb[:], lhsT=wt_t[m][:, bass.ts(kt, 128)],
                                rhs=xb[:, kt, :],
                                start=(kt == 0), stop=(kt == KT - 1))
                        for n, ps in ((2 * ng, psa), (2 * ng + 1, psb)):
                            ob = out_pool.tile([128, NTOK], odt, tag="ob",
                                               name="ob")
                            nc.vector.tensor_scalar_add(
                                ob[:], ps[:], bias_t[:, m:m + 1])
                            nc.sync.dma_start(
                                oview[m, :, bass.ts(n, NTOK)], ob[:])

            def body():
                for n in range(NT):
                    xns = []
                    for s in range(xsplit):
                        xs = x_pool.tile([128, ksub, NTOK], bf16,
                                         tag=f"xn{s}", name=f"xn{s}")
                        nc.sync.dma_start(
                            xs[:],
                            xview[:, bass.ts(s, ksub), bass.ts(n, NTOK)])
                        xns.append(xs)
                    for m in range(MT):
                        ps = psum_pool.tile([128, NTOK], f32, tag="ps")
                        for kt in range(KT):
                            nc.tensor.matmul(
                                ps[:],
                                lhsT=wt_t[m][:, bass.ts(kt, 128)],
                                rhs=xns[kt // ksub][:, kt % ksub, :],
                                start=(kt == 0), stop=(kt == KT - 1),
                            )
                        ob = out_pool.tile([128, NTOK], odt, tag="ob")
                        nc.vector.tensor_scalar_add(
                            ob[:], ps[:], bias_t[:, m:m + 1])
                        nc.sync.dma_start(
                            oview[m, :, bass.ts(n, NTOK)], ob[:])

            run_body = body_wreuse if wreuse else body
            if repeat == 1:
                dequant()
                run_body()
            else:
                with tc.For_i(0, repeat, 1):
                    dequant()
                    run_body()

    nc.compile()
    return nc


def _host_prep(x, centroids, labels, bias, device_gather: bool,
               precision: str = PRECISION):
    """Relayout inputs for the per-core DRAM tensors."""
    lpdt = np.float16 if precision == "f16" else ml_dtypes.bfloat16
    labels2d = np.asarray(labels).reshape(DOUT, DIN // BD)   # [out, block]
    cent = np.asarray(centroids, dtype=np.float32)           # [256, 16]
    # cast before transposing: elementwise astype commutes with .T and the
    # fp16 transpose-copy moves half the bytes of the fp32 one
    xT = np.ascontiguousarray(
        np.asarray(x).reshape(TOK, DIN).astype(lpdt).T)      # [DIN, TOK]

    in_maps = []
    for c in range(N_CORES):
        osl = slice(c * OSH, (c + 1) * OSH)
        bias_c = np.ascontiguousarray(
            np.asarray(bias, dtype=np.float32)[osl].reshape(MT, 128).T)
        m = {"xT": xT, "biasc": bias_c}
        if device_gather:
            # per-partition centroid-column table: row 16b+j = centroids[:, j]
            table = np.ascontiguousarray(np.tile(cent.T, (8, 1)))  # [128, 256]
            # index sequence per group b, consumption order (m, kt, o'):
            # labels2d[osl][m*128+o', 8*kt+b]
            l4 = labels2d[osl].reshape(MT, 128, KT, 8)   # [m, o', kt, b]
            seq = l4.transpose(3, 0, 2, 1).reshape(8, MT * KT * 128)  # [b, i]
            wrapped = seq.reshape(8, MT * KT * 128 // 16, 16)
            idx = np.ascontiguousarray(
                wrapped.transpose(0, 2, 1).reshape(128, MT * KT * 128 // 16)
            ).astype(np.int16)
            m["table"] = table
            m["idx"] = idx
        else:
            # host dequant of the W^T shard in (m, kt, o') free layout:
            # wt[16b+j, m*KT*128 + kt*128 + o'] = cent[labels2d[osl][m*128+o', 8kt+b], j]
            w = cent[labels2d[osl]]                  # [512, 256, 16]
            w = w.reshape(MT, 128, KT, 8, BD)        # [m, o', kt, b, j]
            wt = w.transpose(3, 4, 0, 2, 1).reshape(128, MT * KT * 128)
            m["wt"] = np.ascontiguousarray(wt).astype(lpdt)
        in_maps.append(m)
    return in_maps


_CACHE = {}


def kernel(x, centroids, labels, bias):
    key = (DEVICE_GATHER,)
    if key not in _CACHE:
        _CACHE[key] = build_nc(DEVICE_GATHER, repeat=1)
    nc = _CACHE[key]
    in_maps = _host_prep(x, centroids, labels, bias, DEVICE_GATHER)
    res = bass_utils.run_bass_kernel_spmd(
        nc, in_maps, core_ids=list(range(N_CORES)))
    # assemble [TOK, DOUT] directly: one transposing-cast copy per shard
    # instead of concatenate + full-matrix transpose materialization
    out = np.empty((TOK, DOUT), np.float32)
    for c in range(N_CORES):
        out[:, c * OSH:(c + 1) * OSH] = res.results[c]["out"].T
    return out.reshape(4, 2048, DOUT)



# revision 7
# speedup vs baseline: 11.3836x; 11.3836x over previous
"""VQ-codebook linear layer on 8 Trainium2 NeuronCores.

Computes  out = x @ W^T + bias  where  W = centroids[labels].reshape(4096, 4096).

Strategy (tensor-parallel over out_features, per the sharding hint:
"shard labels/centroid-gathered weight along out_features, replicate x"):
  - each core owns a 512-row slice of W (out_features / 8)
  - x is replicated to every core (transposed to [in, tok] fp16 on host)
  - the centroid-gathered weight shard is either pre-gathered on host
    (DEVICE_GATHER=False, fast) or dequantized on-device on the GPSIMD
    engine via ap_gather from a per-partition centroid-column table
    (DEVICE_GATHER=True; measured ~1 ms slower — ap_gather runs at
    ~60 cycles/index and cannot hide under the matmul stream)
  - matmul in MIXED low precision (fp32 PSUM accumulation): lhsT = W^T
    tile [128 in, 128 out] fp16 (stationary), rhs = x^T tile
    [128 in, 512 tok] bf16 (moving), PSUM accumulates over 32 k-tiles;
    bias added on the PSUM->SBUF copy (DVE tensor_scalar_add).
    Measured on this part, the PE column-stream rate depends on the
    MOVING operand dtype only (N=512: fp16 371 ns/MM, bf16 ~310,
    fp8 270; zero fixed per-MM cost — LDWEIGHTS/drain fully hidden),
    so bf16-x buys ~15% over fp16-x while fp16-W keeps the W-side
    quantization error 4x below all-bf16 (which fails the 2e-2 gate
    at 2.7e-2; mixed measures 1.2e-2, all-fp16 3.3e-3).
  - the kernel is PE-streaming-bound: DMA (x 64 MiB + wt 4 MiB + out
    8 MiB per core per pass, ~315-345 us standalone) hides fully under
    the ~650 us matmul stream. Contiguous-x relayout, scalar-ring
    stores, deeper PSUM/x buffering, and weight-reuse orderings were
    all measured neutral-to-slower.
  - per-core output is [512 out, 8192 tok] fp16; host concatenates and
    transposes back to [4, 2048, 4096]
"""

import numpy as np
import ml_dtypes
import concourse.bass as bass
import concourse.tile as tile
from concourse import bacc, mybir, library_config
from concourse import bass_utils

TOK = 8192          # 4 * 2048 tokens
DIN = 4096
DOUT = 4096
BD = 16             # block dim (centroid vector length)
NCLUST = 256        # codebook size
N_CORES = 8
OSH = DOUT // N_CORES        # 512 out features per core
MT = OSH // 128              # 4 m-tiles per core
KT = DIN // 128              # 32 k-tiles
NTOK = 512                   # tokens per matmul (moving free dim)
NT = TOK // NTOK             # 16 n-tiles

DEVICE_GATHER = False  # host-gathered W^T shard: ~243 us/core vs ~938 us
                       # with on-device ap_gather dequant (measured)
PRECISION = "mixed"    # "f16" | "bf16" | "mixed" — matmul dtypes.
                       # "mixed" = x (moving operand) bf16, W (stationary)
                       # fp16: the PE streams the moving operand at the
                       # moving dtype's rate (bf16 310 ns/MM vs fp16 371
                       # measured on this part), while fp16 weights keep
                       # the W-side quantization error 4x smaller than
                       # all-bf16 (which measured rel err 2.7e-2 > 2e-2).


OUT_F16 = True         # write the [512, 8192] output shard as fp16 (host
                       # upconverts): 8 MiB/core less device DMA and ~2.5 s
                       # less D2H per call; costs ~1.3e-3 extra rel err
XSPLIT = 1             # k-splitting the x n-tile DMA measured slower
                       # (xsplit=2 regressed ~0.6 ms/iter in the A/B); keep 1
WREUSE = False         # issue each W^T k-tile's matmul for 2 token-tiles
                       # back-to-back (halves LDWEIGHTS count)


def build_nc(device_gather: bool = DEVICE_GATHER, repeat: int = 1,
             precision: str = PRECISION, out_f16: bool = OUT_F16,
             xsplit: int = XSPLIT, wreuse: bool = WREUSE,
             psum_bufs: int = 4, x_bufs: int = 2, deq_scalar: bool = False,
             wt_pingpong: bool = False):
    """Build and bacc-compile the per-core bass program (SPMD: all cores run
    the same program on different DRAM inputs)."""
    import contextlib

    nc = bacc.Bacc("TRN2", target_bir_lowering=False, debug=False,
                   enable_asserts=True, num_devices=N_CORES)
    f32, i16 = mybir.dt.float32, mybir.dt.int16
    if precision == "mixed":
        xdt, wdt = mybir.dt.bfloat16, mybir.dt.float16
    elif precision == "bf16":
        xdt = wdt = mybir.dt.bfloat16
    else:
        xdt = wdt = mybir.dt.float16

    xT_ap = nc.dram_tensor("xT", [DIN, TOK], xdt, kind="ExternalInput").ap()
    bias_ap = nc.dram_tensor("biasc", [128, MT], f32, kind="ExternalInput").ap()
    if device_gather:
        table_ap = nc.dram_tensor("table", [128, NCLUST], f32,
                                  kind="ExternalInput").ap()
        idx_ap = nc.dram_tensor("idx", [128, MT * KT * 128 // 16], i16,
                                kind="ExternalInput").ap()
    else:
        wt_ap = nc.dram_tensor("wt", [128, MT * KT * 128], wdt,
                               kind="ExternalInput").ap()
    odt = mybir.dt.float16 if out_f16 else f32
    out_ap = nc.dram_tensor("out", [OSH, TOK], odt, kind="ExternalOutput").ap()

    # DRAM views
    # xT [DIN, TOK] -> [128 p, KT, TOK]
    xview = xT_ap.rearrange("(kt p) t -> p kt t", p=128)
    # out [OSH, TOK] -> [MT, 128 p, TOK]
    oview = out_ap.rearrange("(mt p) t -> mt p t", p=128)

    with tile.TileContext(nc) as tc:
        with contextlib.ExitStack() as ctx:
            const_pool = ctx.enter_context(tc.tile_pool(name="const", bufs=1))
            wt_pool = ctx.enter_context(tc.tile_pool(name="wt", bufs=1))
            x_pool = ctx.enter_context(tc.tile_pool(name="x", bufs=x_bufs))
            psum_pool = ctx.enter_context(
                tc.tile_pool(name="psum", bufs=psum_bufs, space="PSUM"))
            out_pool = ctx.enter_context(tc.tile_pool(name="ob", bufs=3))
            if device_gather:
                g_pool = ctx.enter_context(tc.tile_pool(name="g", bufs=2))
                nc.gpsimd.load_library(library_config.ap_gather)

            bias_t = const_pool.tile([128, MT], f32)
            nc.sync.dma_start(bias_t[:], bias_ap[:])

            # W^T per-core shard, free layout (m, kt, o'): 32 KB/part
            nbank = 2 if (wt_pingpong and repeat > 1) else 1
            wt_banks = [[wt_pool.tile([128, KT * 128], wdt, tag=f"wt{b}_{m}",
                                      name=f"wt{b}_{m}")
                         for m in range(MT)] for b in range(nbank)]
            wt_t = wt_banks[0]

            if device_gather:
                table_t = const_pool.tile([128, NCLUST], f32)
                nc.sync.dma_start(table_t[:], table_ap[:])
                idx_t = const_pool.tile([128, MT * KT * 128 // 16], i16)
                nc.sync.dma_start(idx_t[:], idx_ap[:])

            def dequant(bank=0):
                wt_t = wt_banks[bank]
                if device_gather:
                    nidx_chunk = KT * 128  # 4096 per 16-part group per m-chunk
                    for m in range(MT):
                        g = g_pool.tile([128, nidx_chunk], f32, tag="g")
                        nc.gpsimd.ap_gather(
                            g[:], table_t[:],
                            idx_t[:, bass.ts(m, nidx_chunk // 16)],
                            channels=128, num_elems=NCLUST, d=1,
                            num_idxs=nidx_chunk,
                        )
                        nc.vector.tensor_copy(wt_t[m][:], g[:])
                else:
                    deq_eng = nc.scalar if deq_scalar else nc.sync
                    for m in range(MT):
                        deq_eng.dma_start(wt_t[m][:],
                                          wt_ap[:, bass.ts(m, KT * 128)])

            ksub = KT // xsplit

            def run_body_bank(bank):
                nonlocal wt_t
                wt_t = wt_banks[bank]
                (body_wreuse if wreuse else body)()

            def body_wreuse():
                # pairs of token-tiles share each loaded W^T k-tile: the two
                # matmuls after one implicit LDWEIGHTS reuse the stationary
                # operand, halving weight-load pressure on the PE
                for ng in range(NT // 2):
                    xa = x_pool.tile([128, KT, NTOK], xdt, tag="xn0",
                                     name="xa")
                    nc.sync.dma_start(xa[:], xview[:, :, bass.ts(2 * ng, NTOK)])
                    xb = x_pool.tile([128, KT, NTOK], xdt, tag="xn1",
                                     name="xb")
                    nc.sync.dma_start(xb[:], xview[:, :, bass.ts(2 * ng + 1, NTOK)])
                    for m in range(MT):
                        psa = psum_pool.tile([128, NTOK], f32, tag="ps")
                        psb = psum_pool.tile([128, NTOK], f32, tag="ps")
                        for kt in range(KT):
                            nc.tensor.matmul(
                                psa[:], lhsT=wt_t[m][:, bass.ts(kt, 128)],
                                rhs=xa[:, kt, :],
                                start=(kt == 0), stop=(kt == KT - 1))
                            nc.tensor.matmul(
                                psb[:], lhsT=wt_t[m][:, bass.ts(kt, 128)],
                                rhs=xb[:, kt, :],
                                start=(kt == 0), stop=(kt == KT - 1))
                        for n, ps in ((2 * ng, psa), (2 * ng + 1, psb)):
                            ob = out_pool.tile([128, NTOK], odt, tag="ob",
                                               name="ob")
                            nc.vector.tensor_scalar_add(
                                ob[:], ps[:], bias_t[:, m:m + 1])
                            nc.sync.dma_start(
                                oview[m, :, bass.ts(n, NTOK)], ob[:])

            def body():
                for n in range(NT):
                    xns = []
                    for s in range(xsplit):
                        xs = x_pool.tile([128, ksub, NTOK], xdt,
                                         tag=f"xn{s}", name=f"xn{s}")
                        nc.sync.dma_start(
                            xs[:],
                            xview[:, bass.ts(s, ksub), bass.ts(n, NTOK)])
                        xns.append(xs)
                    for m in range(MT):
                        ps = psum_pool.tile([128, NTOK], f32, tag="ps")
                        for kt in range(KT):
                            nc.tensor.matmul(
                                ps[:],
                                lhsT=wt_t[m][:, bass.ts(kt, 128)],
                                rhs=xns[kt // ksub][:, kt % ksub, :],
                                start=(kt == 0), stop=(kt == KT - 1),
                            )
                        ob = out_pool.tile([128, NTOK], odt, tag="ob")
                        nc.vector.tensor_scalar_add(
                            ob[:], ps[:], bias_t[:, m:m + 1])
                        nc.sync.dma_start(
                            oview[m, :, bass.ts(n, NTOK)], ob[:])

            run_body = body_wreuse if wreuse else body
            if repeat == 1:
                dequant()
                run_body()
            elif wt_pingpong:
                # two weight banks ping-pong across passes: bank b loads
                # while the body computes from bank 1-b, hiding the per-pass
                # dequant DMA behind compute. repeat must be even.
                assert repeat % 2 == 0
                dequant(0)
                with tc.For_i(0, repeat // 2, 1):
                    dequant(1)
                    run_body_bank(0)
                    dequant(0)
                    run_body_bank(1)
            else:
                with tc.For_i(0, repeat, 1):
                    dequant()
                    run_body()

    nc.compile()
    return nc


def _host_prep(x, centroids, labels, bias, device_gather: bool,
               precision: str = PRECISION):
    """Relayout inputs for the per-core DRAM tensors."""
    if precision == "mixed":
        lpdt, wpdt = ml_dtypes.bfloat16, np.float16
    elif precision == "bf16":
        lpdt = wpdt = ml_dtypes.bfloat16
    else:
        lpdt = wpdt = np.float16
    labels2d = np.asarray(labels).reshape(DOUT, DIN // BD)   # [out, block]
    cent = np.asarray(centroids, dtype=np.float32)           # [256, 16]
    # cast before transposing: elementwise astype commutes with .T and the
    # fp16 transpose-copy moves half the bytes of the fp32 one
    xT = np.ascontiguousarray(
        np.asarray(x).reshape(TOK, DIN).astype(lpdt).T)      # [DIN, TOK]

    in_maps = []
    for c in range(N_CORES):
        osl = slice(c * OSH, (c + 1) * OSH)
        bias_c = np.ascontiguousarray(
            np.asarray(bias, dtype=np.float32)[osl].reshape(MT, 128).T)
        m = {"xT": xT, "biasc": bias_c}
        if device_gather:
            # per-partition centroid-column table: row 16b+j = centroids[:, j]
            table = np.ascontiguousarray(np.tile(cent.T, (8, 1)))  # [128, 256]
            # index sequence per group b, consumption order (m, kt, o'):
            # labels2d[osl][m*128+o', 8*kt+b]
            l4 = labels2d[osl].reshape(MT, 128, KT, 8)   # [m, o', kt, b]
            seq = l4.transpose(3, 0, 2, 1).reshape(8, MT * KT * 128)  # [b, i]
            wrapped = seq.reshape(8, MT * KT * 128 // 16, 16)
            idx = np.ascontiguousarray(
                wrapped.transpose(0, 2, 1).reshape(128, MT * KT * 128 // 16)
            ).astype(np.int16)
            m["table"] = table
            m["idx"] = idx
        else:
            # host dequant of the W^T shard in (m, kt, o') free layout:
            # wt[16b+j, m*KT*128 + kt*128 + o'] = cent[labels2d[osl][m*128+o', 8kt+b], j]
            w = cent[labels2d[osl]]                  # [512, 256, 16]
            w = w.reshape(MT, 128, KT, 8, BD)        # [m, o', kt, b, j]
            wt = w.transpose(3, 4, 0, 2, 1).reshape(128, MT * KT * 128)
            m["wt"] = np.ascontiguousarray(wt).astype(wpdt)
        in_maps.append(m)
    return in_maps


_CACHE = {}


def kernel(x, centroids, labels, bias):
    key = (DEVICE_GATHER,)
    if key not in _CACHE:
        _CACHE[key] = build_nc(DEVICE_GATHER, repeat=1)
    nc = _CACHE[key]
    in_maps = _host_prep(x, centroids, labels, bias, DEVICE_GATHER)
    res = bass_utils.run_bass_kernel_spmd(
        nc, in_maps, core_ids=list(range(N_CORES)))
    # assemble [TOK, DOUT] directly: one transposing-cast copy per shard
    # instead of concatenate + full-matrix transpose materialization
    out = np.empty((TOK, DOUT), np.float32)
    for c in range(N_CORES):
        out[:, c * OSH:(c + 1) * OSH] = res.results[c]["out"].T
    return out.reshape(4, 2048, DOUT)

